# revision 15
# baseline (speedup 1.0000x reference)
"""Multi-head attention kernel for Trainium2 (8 NeuronCores, data-parallel over batch).

Reference computation (per batch b of 8):
    x:  [1024, 768]  (tokens x channels, n = 32*32)
    qkv = x @ qkv_w.T                    -> [1024, 2304]
    q, k, v per head (12 heads, dh=64)
    S = q @ k.T * dh**-0.5; P = softmax(S); O = P @ v
    out = concat_heads(O) @ proj_w.T + proj_b
Each core processes one batch element independently (no collectives).

Inputs are pre-transposed and pre-cast to bf16 on the host (a free layout
choice: the DRAM tensors are declared in c-major tile order), so SBUF
operands stream straight in with no on-chip transposes or cast staging:
    xT_d      [768, 1024]      x^T rows = channels
    qkv_wt_d  [18, 128, 768]   tile ot: qkv_w[ot*128:+128, :].T blocked by ct
    proj_wt_d [6, 128, 768]    same for proj_w

On-chip layouts (bf16 compute, fp32 PSUM accumulation):
    x_all     [128c, 6ct, 1024t]          (x^T: c on partitions)
    wq_all    [128c, 6ct, 2304o]          (qkv_w^T)
    wp_all    [128c, 6ct, 768o]           (proj_w^T)
    qkT[i]    [128o, 1024t]  i=0..11      (q^T tiles 0-5, k^T tiles 6-11)
    V[tt]     [128t, 12h, 65]             (v natural + ones column per head)
    E[h]      [128j, 8jt, 1024i]          (exp(S^T) per head, bf16)
    Onat      [128i, 2h, 64d]  per i-tile (normalized attention out, natural)
    OT[g]     [128c, 1024t]  g=0..5       (attention out transposed, head pairs)

Attention per head (transposed scores, no max subtraction - scores are O(1)
for this distribution and exp runs in fp32):
    S^T[j,i] = sum_d k^T[d,j] q^T[d,i]       (matmul, K=dh=64, head pairs
                                              row-packed on the PE array)
    E^T = exp(S^T * scale)                    (ACT, PSUM->SBUF, bf16)
    [O | denom] = E^T.T @ [V|1]               (matmul per i-tile: M=128 i,
                                              K=128 j, N=65 - full PE rate,
                                              2.2x fewer PE cycles than the
                                              M=65 transposed form)
    O /= denom                                (DVE reciprocal + per-partition
                                              tensor_scalar multiply)
    OT = O^T                                  (PE transpose vs identity, then
                                              DVE copy PSUM->SBUF)

Emission is a flat interleaved schedule: the PE queue is in-order, so S-unit
fills (paced by the ACT exp stream via 2 PSUM bufs), next-pair QKV chains,
O accumulations of the previous pair, and O transposes are woven so the PE
always has ready work while ACT streams exp.
"""

import numpy as np

import concourse.bass as bass
import concourse.mybir as mybir
import concourse.tile as tile
from concourse import bacc
from concourse.masks import make_identity

# Problem constants (hardcoded per contract)
B = 8
N = 1024          # tokens per batch (32*32)
C = 768           # channels
H = 12            # heads
DH = 64           # head dim
O3 = 3 * C        # 2304
SCALE = DH ** -0.5
NCORES = 8

F32 = mybir.dt.float32
BF16 = mybir.dt.bfloat16

CT = C // 128     # 6 c-tiles
TT = N // 128     # 8 token tiles
IC = N // 512     # 2 i-chunks of 512
JT = N // 128     # 8 j-tiles
WT = O3 // 128    # 18 qkv_w o-tiles


def _build_nc(dbg=False, repeat=1):
    nc = bacc.Bacc("TRN2", target_bir_lowering=False, debug=False, num_devices=NCORES)

    xt_d = nc.dram_tensor("xT", [C, N], BF16, kind="ExternalInput").ap()
    qkvwt_d = nc.dram_tensor("qkv_wt", [WT, 128, C], BF16, kind="ExternalInput").ap()
    projwt_d = nc.dram_tensor("proj_wt", [CT, 128, C], BF16, kind="ExternalInput").ap()
    projb_d = nc.dram_tensor("proj_b", [C], F32, kind="ExternalInput").ap()
    out_d = nc.dram_tensor("out", [N, C], F32, kind="ExternalOutput").ap()

    with tile.TileContext(nc) as tc:
        _emit(nc, tc, xt_d, qkvwt_d, projwt_d, projb_d, out_d, repeat=repeat)
    nc.compile()
    return nc


def _emit(nc, tc, xt_d, qkvwt_d, projwt_d, projb_d, out_d, repeat=1):
    from contextlib import ExitStack

    with ExitStack() as ctx:
        # ---------------- pools ----------------
        sb = lambda name, bufs: ctx.enter_context(tc.tile_pool(name=name, bufs=bufs))
        ps = lambda name, bufs: ctx.enter_context(
            tc.tile_pool(name=name, bufs=bufs, space="PSUM")
        )

        big_pool = sb("big", 1)          # x_all / wq_all / wp_all / identity
        qkT_pool = sb("qkT", 12)
        v_pool = sb("vbf", TT)
        e_pool = sb("ebf", 4)            # two pairs of E tiles in flight
        ot_sb_pool = sb("otsb", CT)
        onat_pool = sb("onat", 16)
        pjp_pool = sb("pjpart", TT)
        rec_pool = sb("rec", 8)
        bias_pool = sb("bias", 1)
        out_pool = sb("outsb", 3)

        qkv_ps = ps("qkvps", 2)          # 1 bank each: QKV + proj accs
        sps_ps = ps("sps", 2)            # 2 banks each: S^T units (1 head)
        o_ps = ps("ops", 2)              # 1 bank each: O accs + transposes

        # ---------------- persistent tiles ----------------
        ident = big_pool.tile([128, 128], BF16, tag="ident")
        make_identity(nc, ident)

        x_all = big_pool.tile([128, CT, N], BF16, tag="x_all")
        wq_all = big_pool.tile([128, CT, O3], BF16, tag="wq_all")
        wp_all = big_pool.tile([128, CT, C], BF16, tag="wp_all")

        qkT = [
            qkT_pool.tile([128, N], BF16, tag="qkT", name=f"qkT_{i}") for i in range(12)
        ]
        Vt = [
            v_pool.tile([128, H, DH + 1], BF16, tag="vbf", name=f"V_{i}")
            for i in range(TT)
        ]
        OT = [
            ot_sb_pool.tile([128, N], BF16, tag="otsb", name=f"OT_{i}")
            for i in range(CT)
        ]

        # bias broadcast to all partitions (fp32); emitted via the schedule so
        # its slow single-partition DMA never delays the x/w input stream
        bias_row = bias_pool.tile([1, C], F32, tag="biasrow")
        bias_bc = bias_pool.tile([128, C], F32, tag="biasbc")

        def load_bias():
            nc.scalar.dma_start(out=bias_row, in_=projb_d[None, :])
            nc.gpsimd.partition_broadcast(bias_bc, bias_row)

        # ---------------- micro-emitters ----------------
        qi = 0

        def _eng():
            nonlocal qi
            qi += 1
            return nc.gpsimd if qi % 2 else nc.sync

        # x rides 3 queues so the first QKV chain is paced by compute, not DMA
        x_engs = [nc.sync, nc.gpsimd, nc.scalar]

        def load_x(rt):
            x_engs[rt % 3].dma_start(
                out=x_all[:, rt, :], in_=xt_d[rt * 128:(rt + 1) * 128, :]
            )

        def load_w(ot):
            _eng().dma_start(
                out=wq_all[:, :, ot * 128:(ot + 1) * 128], in_=qkvwt_d[ot]
            )

        def load_wp(ot):
            _eng().dma_start(
                out=wp_all[:, :, ot * 128:(ot + 1) * 128], in_=projwt_d[ot]
            )

        def qk_chain(g, which):
            # which: 0 = q ic0, 1 = k ic0, 2 = q ic1, 3 = k ic1
            ic, isk = which >> 1, which & 1
            obase = (C if isk else 0) + g * 128
            dst = qkT[(6 if isk else 0) + g]
            acc = qkv_ps.tile([128, 512], F32, tag="qkvps", name="qk_acc")
            for ct in range(CT):
                nc.tensor.matmul(
                    acc,
                    lhsT=wq_all[:, ct, obase:obase + 128],
                    rhs=x_all[:, ct, ic * 512:(ic + 1) * 512],
                    start=(ct == 0),
                    stop=(ct == CT - 1),
                )
            nc.vector.tensor_copy(dst[:, ic * 512:(ic + 1) * 512], acc)

        def v_chain(oc, tt):
            # v rows o in [1536 + oc*384, +384) -> heads 6*oc .. 6*oc+5
            acc = qkv_ps.tile([128, 384], F32, tag="qkvps", name="v_acc")
            for ct in range(CT):
                nc.tensor.matmul(
                    acc,
                    lhsT=x_all[:, ct, tt * 128:(tt + 1) * 128],
                    rhs=wq_all[:, ct, 2 * C + oc * 384:2 * C + (oc + 1) * 384],
                    start=(ct == 0),
                    stop=(ct == CT - 1),
                )
            if oc == 0:
                nc.vector.memset(Vt[tt][:, :, DH:DH + 1], 1.0)
            nc.vector.tensor_copy(
                Vt[tt][:, 6 * oc:6 * (oc + 1), 0:DH],
                acc.rearrange("p (h d) -> p h d", d=DH),
            )

        E = {}      # pair -> {h: tile}
        onat = {}   # pair -> {it: tile}

        def s_unit(g, ic, u, h):
            # 2 j-tiles of S^T for one head + one exp instr
            if g not in E:
                E[g] = {
                    hh: e_pool.tile([128, JT, N], BF16, tag="ebf", name=f"E_{hh}")
                    for hh in (2 * g, 2 * g + 1)
                }
            hoff = (h % 2) * DH
            un = sps_ps.tile([128, 2, 512], F32, tag="sps", name="sT")
            for q in range(2):
                jt = 2 * u + q
                nc.tensor.matmul(
                    un[:, q, :],
                    lhsT=qkT[6 + g][hoff:hoff + DH, jt * 128:(jt + 1) * 128],
                    rhs=qkT[g][hoff:hoff + DH, ic * 512:(ic + 1) * 512],
                    start=True,
                    stop=True,
                )
            nc.scalar.activation(
                E[g][h][:, 2 * u:2 * u + 2, ic * 512:(ic + 1) * 512],
                un,
                mybir.ActivationFunctionType.Exp,
                scale=SCALE,
            )

        def o_acc(g, it):
            # natural-layout O for one i-tile, both heads: full PE rate
            h0, h1 = 2 * g, 2 * g + 1
            acc = o_ps.tile([128, 2, DH + 1], F32, tag="ops", name="o_acc")
            for h in (h0, h1):
                c = h % 2
                for jt in range(JT):
                    nc.tensor.matmul(
                        acc[:, c, :],
                        lhsT=E[g][h][:, jt, it * 128:(it + 1) * 128],
                        rhs=Vt[jt][:, h, :],
                        start=(jt == 0),
                        stop=(jt == JT - 1),
                    )
            rec = rec_pool.tile([128, 2], F32, tag="rec", name="rec")
            nc.vector.reciprocal(rec, acc[:, :, DH])
            ona = onat_pool.tile([128, 2, DH], BF16, tag="onat", name="ona")
            for c in range(2):
                nc.vector.tensor_scalar_mul(
                    ona[:, c, :], acc[:, c, 0:DH], rec[:, c:c + 1]
                )
            onat.setdefault(g, {})[it] = ona

        def o_tr(g, it):
            # transpose [128 i, 128 hd] -> OT[g] columns for this i-tile
            tp = o_ps.tile([128, 128], BF16, tag="ops", name="tp")
            nc.tensor.transpose(
                tp, onat[g][it].rearrange("p a b -> p (a b)"), ident
            )
            nc.vector.tensor_copy(OT[g][:, it * 128:(it + 1) * 128], tp)
            if it == TT - 1:
                del onat[g]
                del E[g]

        pj_partial = [
            pjp_pool.tile([128, C], BF16, tag="pjpart", name=f"pjp_{i}")
            for i in range(TT)
        ]

        def pj1(tt):
            # head pairs 0-4: overlaps the ACT-bound tail of attention pair 5
            for oc in range(2):
                acc = qkv_ps.tile([128, 384], F32, tag="qkvps", name="pj_acc")
                for g in range(5):
                    nc.tensor.matmul(
                        acc,
                        lhsT=OT[g][:, tt * 128:(tt + 1) * 128],
                        rhs=wp_all[:, g, oc * 384:(oc + 1) * 384],
                        start=(g == 0),
                        stop=(g == 4),
                    )
                nc.vector.tensor_add(
                    pj_partial[tt][:, oc * 384:(oc + 1) * 384],
                    acc,
                    bias_bc[:, oc * 384:(oc + 1) * 384],
                )

        def pj2(tt):
            # head pair 5 only - the thinnest possible serial tail
            osb = out_pool.tile([128, C], F32, tag="outsb", name="osb")
            for oc in range(2):
                acc = qkv_ps.tile([128, 384], F32, tag="qkvps", name="pj_acc")
                nc.tensor.matmul(
                    acc,
                    lhsT=OT[5][:, tt * 128:(tt + 1) * 128],
                    rhs=wp_all[:, 5, oc * 384:(oc + 1) * 384],
                    start=True,
                    stop=True,
                )
                nc.vector.tensor_add(
                    osb[:, oc * 384:(oc + 1) * 384],
                    acc,
                    pj_partial[tt][:, oc * 384:(oc + 1) * 384],
                )
                nc.sync.dma_start(
                    out=out_d[tt * 128:(tt + 1) * 128, oc * 384:(oc + 1) * 384],
                    in_=osb[:, oc * 384:(oc + 1) * 384],
                )

        # ---------------- emission schedule ----------------
        # Steady-state slot p (pairs 1..4): S fills of pair p (ACT-paced via
        # the 2 sps bufs), O ic1 of pair p-1, QKV chains of pair p+1, O ic0
        # of pair p, transposes of pair p-1 - all woven at unit granularity.
        def s_units(g, ic):
            return [(s_unit, g, ic, u, h) for u in range(4) for h in (2 * g, 2 * g + 1)]

        def weave(a, b):
            # interleave pacer list a with filler list b, spreading b evenly
            # (fractional accumulator) so no burst of fillers stalls on PSUM
            out = []
            ib = 0
            acc_f = 0.0
            step = len(b) / max(1, len(a))
            for ia, item in enumerate(a):
                out.append(item)
                acc_f += step
                while ib < round(acc_f):
                    out.append(b[ib]); ib += 1
            out.extend(b[ib:])
            return out

        for _ in range(repeat):
            sched = []
            # startup: pair-0 weights and x in parallel across the DMA queues
            sched += [(load_x, 0)]
            sched += [(load_w, 0)]
            sched += [(load_x, rt) for rt in range(1, CT)]
            sched += [(load_w, 6), (load_w, 12), (load_w, 13), (load_w, 14)]
            sched += [(load_bias,)]
            sched += [(qk_chain, 0, w) for w in range(4)]
            # pair-0 S fills woven with v chains (exp(0) streams on ACT)
            sched += weave(
                s_units(0, 0) + s_units(0, 1),
                [(v_chain, 0, tt) for tt in range(TT)]
                + [(load_w, 1), (load_w, 7)]
                + [(qk_chain, 1, w) for w in range(4)]
                + [(o_acc, 0, i) for i in range(4)],
            )
            # slots for pairs 1..4
            for p in range(1, 5):
                filler = [(o_acc, p - 1, 4 + i) for i in range(4)]
                filler += [(o_tr, p - 1, i) for i in range(4)]
                if p == 2:
                    filler += [(load_w, 15), (load_w, 16), (load_w, 17)]
                filler += [(load_w, p + 1), (load_w, 7 + p)]
                filler += [(qk_chain, p + 1, w) for w in range(4)]
                if p == 4:
                    filler += [(load_wp, rt) for rt in range(CT)]
                if p == 3:
                    filler += [(v_chain, 1, tt) for tt in range(TT)]
                filler += [(o_acc, p, i) for i in range(4)]
                filler += [(o_tr, p - 1, 4 + i) for i in range(4)]
                sched += weave(s_units(p, 0) + s_units(p, 1), filler)
            # slot for pair 5: S fills + O(4) + proj pass 1
            filler = [(o_acc, 4, 4 + i) for i in range(4)]
            filler += [(o_tr, 4, i) for i in range(8)]
            filler += [(pj1, tt) for tt in range(TT)]
            filler += [(o_acc, 5, i) for i in range(4)]
            sched += weave(s_units(5, 0) + s_units(5, 1), filler)
            # tail: weave O ic1 of pair 5 between the ready ic0 transposes and
            # proj-pass-2 tiles so every engine always has a ready item
            for it in range(TT):
                sched += [(o_tr, 5, it), (pj2, it)]
                if it < 4:
                    sched += [(o_acc, 5, 4 + it)]

            for item in sched:
                item[0](*item[1:])


_NC_CACHE = None


def _get_nc():
    global _NC_CACHE
    if _NC_CACHE is None:
        _NC_CACHE = _build_nc()
    return _NC_CACHE


def _prep_in_maps(x, qkv_w, proj_w, proj_b):
    """Host-side shard + pre-transpose + bf16 cast. Returns per-core in_maps."""
    import ml_dtypes

    BF = ml_dtypes.bfloat16
    x = np.ascontiguousarray(np.asarray(x, dtype=np.float32))
    qkv_w = np.asarray(qkv_w, dtype=np.float32)
    proj_w = np.asarray(proj_w, dtype=np.float32)
    proj_b = np.ascontiguousarray(np.asarray(proj_b, dtype=np.float32))

    xf = x.reshape(B, N, C)
    xt = np.ascontiguousarray(xf.transpose(0, 2, 1).astype(BF))          # [B, C, N]
    qkv_wt = np.ascontiguousarray(
        qkv_w.reshape(WT, 128, CT, 128).transpose(0, 3, 2, 1).astype(BF)
    ).reshape(WT, 128, C)
    proj_wt = np.ascontiguousarray(
        proj_w.reshape(CT, 128, CT, 128).transpose(0, 3, 2, 1).astype(BF)
    ).reshape(CT, 128, C)
    return [
        {"xT": xt[i], "qkv_wt": qkv_wt, "proj_wt": proj_wt, "proj_b": proj_b}
        for i in range(NCORES)
    ]


def kernel(x, qkv_w, proj_w, proj_b, _trace=False):
    from concourse.bass_utils import run_bass_kernel_spmd

    in_maps = _prep_in_maps(x, qkv_w, proj_w, proj_b)
    nc = _get_nc()
    res = run_bass_kernel_spmd(nc, in_maps, core_ids=list(range(NCORES)), trace=_trace)
    out = np.stack([r["out"] for r in res.results], axis=0).reshape(B, 32, 32, C)
    if _trace:
        kernel._last_results = res
    return out


# revision 23
# speedup vs baseline: 1.5562x; 1.5562x over previous
"""Multi-head attention kernel for Trainium2 (8 NeuronCores, data-parallel over batch).

Reference computation (per batch b of 8):
    x:  [1024, 768]  (tokens x channels, n = 32*32)
    qkv = x @ qkv_w.T                    -> [1024, 2304]
    q, k, v per head (12 heads, dh=64)
    S = q @ k.T * dh**-0.5; P = softmax(S); O = P @ v
    out = concat_heads(O) @ proj_w.T + proj_b
Each core processes one batch element independently (no collectives).

Inputs are pre-transposed and pre-cast to bf16 on the host (a free layout
choice: the DRAM tensors are declared in c-major tile order), so SBUF
operands stream straight in with no on-chip transposes or cast staging:
    xT_d      [768, 1024]      x^T rows = channels
    qkv_wt_d  [18, 128, 768]   tile ot: qkv_w[ot*128:+128, :].T blocked by ct
    proj_wt_d [6, 128, 768]    same for proj_w

On-chip layouts (bf16 compute, fp32 PSUM accumulation):
    x_all     [128c, 6ct, 1024t]          (x^T: c on partitions)
    wq_all    [128c, 6ct, 2304o]          (qkv_w^T)
    wp_all    [128c, 6ct, 768o]           (proj_w^T)
    qkT[i]    [128o, 1024t]  i=0..11      (q^T tiles 0-5, k^T tiles 6-11)
    V[tt]     [128t, 12h, 65]             (v natural + ones column per head)
    E[h]      [128j, 8jt, 1024i]          (exp(S^T) per head, bf16)
    Onat      [128i, 2h, 64d]  per i-tile (normalized attention out, natural)
    OT[g]     [128c, 1024t]  g=0..5       (attention out transposed, head pairs)

Attention per head (transposed scores, no max subtraction - scores are O(1)
for this distribution and exp runs in fp32):
    S^T[j,i] = sum_d k^T[d,j] q^T[d,i]       (matmul, K=dh=64, head pairs
                                              row-packed on the PE array)
    E^T = exp(S^T * scale)                    (ACT, PSUM->SBUF, bf16)
    [O | denom] = E^T.T @ [V|1]               (matmul per i-tile: M=128 i,
                                              K=128 j, N=65 - full PE rate,
                                              2.2x fewer PE cycles than the
                                              M=65 transposed form)
    O /= denom                                (DVE reciprocal + per-partition
                                              tensor_scalar multiply)
    OT = O^T                                  (PE transpose vs identity, then
                                              DVE copy PSUM->SBUF)

Emission is a flat interleaved schedule: the PE queue is in-order, so S-unit
fills (paced by the ACT exp stream via 2 PSUM bufs), next-pair QKV chains,
O accumulations of the previous pair, and O transposes are woven so the PE
always has ready work while ACT streams exp.
"""

import numpy as np

import concourse.bass as bass
import concourse.mybir as mybir
import concourse.tile as tile
from concourse import bacc
from concourse.masks import make_identity

# Problem constants (hardcoded per contract)
B = 8
N = 1024          # tokens per batch (32*32)
C = 768           # channels
H = 12            # heads
DH = 64           # head dim
O3 = 3 * C        # 2304
SCALE = DH ** -0.5
NCORES = 8

F32 = mybir.dt.float32
BF16 = mybir.dt.bfloat16

CT = C // 128     # 6 c-tiles
TT = N // 128     # 8 token tiles
IC = N // 512     # 2 i-chunks of 512
JT = N // 128     # 8 j-tiles
WT = O3 // 128    # 18 qkv_w o-tiles


def _build_nc(dbg=False, repeat=1):
    nc = bacc.Bacc("TRN2", target_bir_lowering=False, debug=False, num_devices=NCORES)

    xt_d = nc.dram_tensor("xT", [C, N], BF16, kind="ExternalInput").ap()
    qkvwt_d = nc.dram_tensor("qkv_wt", [WT, 128, C], BF16, kind="ExternalInput").ap()
    projwt_d = nc.dram_tensor("proj_wt", [CT, 128, C], BF16, kind="ExternalInput").ap()
    projb_d = nc.dram_tensor("proj_b", [C], F32, kind="ExternalInput").ap()
    out_d = nc.dram_tensor("out", [N, C], F32, kind="ExternalOutput").ap()

    with tile.TileContext(nc) as tc:
        _emit(nc, tc, xt_d, qkvwt_d, projwt_d, projb_d, out_d, repeat=repeat)
    nc.compile()
    return nc


def _emit(nc, tc, xt_d, qkvwt_d, projwt_d, projb_d, out_d, repeat=1):
    from contextlib import ExitStack

    with ExitStack() as ctx:
        # ---------------- pools ----------------
        sb = lambda name, bufs: ctx.enter_context(tc.tile_pool(name=name, bufs=bufs))
        ps = lambda name, bufs: ctx.enter_context(
            tc.tile_pool(name=name, bufs=bufs, space="PSUM")
        )

        big_pool = sb("big", 1)          # x_all / wq_all / wp_all / identity
        qkT_pool = sb("qkT", 12)
        v_pool = sb("vbf", TT)
        e_pool = sb("ebf", 4)            # two pairs of E tiles in flight
        ot_sb_pool = sb("otsb", CT)
        onat_pool = sb("onat", 16)
        pjp_pool = sb("pjpart", TT)
        rec_pool = sb("rec", 8)
        bias_pool = sb("bias", 1)
        out_pool = sb("outsb", 3)

        qkv_ps = ps("qkvps", 2)          # 1 bank each: QKV + proj accs
        sps_ps = ps("sps", 2)            # 2 banks each: S^T units (1 head)
        o_ps = ps("ops", 2)              # 1 bank each: O accs + transposes

        # ---------------- persistent tiles ----------------
        ident = big_pool.tile([128, 128], BF16, tag="ident")
        make_identity(nc, ident)

        x_all = big_pool.tile([128, CT, N], BF16, tag="x_all")
        wq_all = big_pool.tile([128, CT, O3], BF16, tag="wq_all")
        wp_all = big_pool.tile([128, CT, C], BF16, tag="wp_all")

        qkT = [
            qkT_pool.tile([128, N], BF16, tag="qkT", name=f"qkT_{i}") for i in range(12)
        ]
        Vt = [
            v_pool.tile([128, H, DH + 1], BF16, tag="vbf", name=f"V_{i}")
            for i in range(TT)
        ]
        OT = [
            ot_sb_pool.tile([128, N], BF16, tag="otsb", name=f"OT_{i}")
            for i in range(CT)
        ]

        # bias broadcast to all partitions (fp32); emitted via the schedule so
        # its slow single-partition DMA never delays the x/w input stream
        bias_row = bias_pool.tile([1, C], F32, tag="biasrow")
        bias_bc = bias_pool.tile([128, C], F32, tag="biasbc")

        def load_bias():
            nc.scalar.dma_start(out=bias_row, in_=projb_d[None, :])
            nc.gpsimd.partition_broadcast(bias_bc, bias_row)

        # ---------------- micro-emitters ----------------
        qi = 0

        def _eng():
            nonlocal qi
            qi += 1
            return nc.gpsimd if qi % 2 else nc.sync

        # x rides 3 queues so the first QKV chain is paced by compute, not DMA
        x_engs = [nc.sync, nc.gpsimd, nc.scalar]

        def load_x(rt):
            x_engs[rt % 3].dma_start(
                out=x_all[:, rt, :], in_=xt_d[rt * 128:(rt + 1) * 128, :]
            )

        def load_w(ot):
            _eng().dma_start(
                out=wq_all[:, :, ot * 128:(ot + 1) * 128], in_=qkvwt_d[ot]
            )

        def load_wp(ot):
            _eng().dma_start(
                out=wp_all[:, :, ot * 128:(ot + 1) * 128], in_=projwt_d[ot]
            )

        def qk_chain(g, which):
            # which: 0 = q ic0, 1 = k ic0, 2 = q ic1, 3 = k ic1
            ic, isk = which >> 1, which & 1
            obase = (C if isk else 0) + g * 128
            dst = qkT[(6 if isk else 0) + g]
            acc = qkv_ps.tile([128, 512], F32, tag="qkvps", name="qk_acc")
            for ct in range(CT):
                nc.tensor.matmul(
                    acc,
                    lhsT=wq_all[:, ct, obase:obase + 128],
                    rhs=x_all[:, ct, ic * 512:(ic + 1) * 512],
                    start=(ct == 0),
                    stop=(ct == CT - 1),
                )
            nc.vector.tensor_copy(dst[:, ic * 512:(ic + 1) * 512], acc)

        def v_chain(oc, tt):
            # v rows o in [1536 + oc*384, +384) -> heads 6*oc .. 6*oc+5
            acc = qkv_ps.tile([128, 384], F32, tag="qkvps", name="v_acc")
            for ct in range(CT):
                nc.tensor.matmul(
                    acc,
                    lhsT=x_all[:, ct, tt * 128:(tt + 1) * 128],
                    rhs=wq_all[:, ct, 2 * C + oc * 384:2 * C + (oc + 1) * 384],
                    start=(ct == 0),
                    stop=(ct == CT - 1),
                )
            if oc == 0:
                nc.vector.memset(Vt[tt][:, :, DH:DH + 1], 1.0)
            nc.vector.tensor_copy(
                Vt[tt][:, 6 * oc:6 * (oc + 1), 0:DH],
                acc.rearrange("p (h d) -> p h d", d=DH),
            )

        E = {}      # pair -> {h: tile}
        onat = {}   # pair -> {it: tile}

        def s_unit(g, ic, u, h):
            # 2 j-tiles of S^T for one head + one exp instr
            if g not in E:
                E[g] = {
                    hh: e_pool.tile([128, JT, N], BF16, tag="ebf", name=f"E_{hh}")
                    for hh in (2 * g, 2 * g + 1)
                }
            hoff = (h % 2) * DH
            un = sps_ps.tile([128, 2, 512], F32, tag="sps", name="sT")
            for q in range(2):
                jt = 2 * u + q
                nc.tensor.matmul(
                    un[:, q, :],
                    lhsT=qkT[6 + g][hoff:hoff + DH, jt * 128:(jt + 1) * 128],
                    rhs=qkT[g][hoff:hoff + DH, ic * 512:(ic + 1) * 512],
                    start=True,
                    stop=True,
                )
            nc.scalar.activation(
                E[g][h][:, 2 * u:2 * u + 2, ic * 512:(ic + 1) * 512],
                un,
                mybir.ActivationFunctionType.Exp,
                scale=SCALE,
            )

        def o_acc(g, it):
            # natural-layout O for one i-tile, both heads: full PE rate
            h0, h1 = 2 * g, 2 * g + 1
            acc = o_ps.tile([128, 2, DH + 1], F32, tag="ops", name="o_acc")
            for h in (h0, h1):
                c = h % 2
                for jt in range(JT):
                    nc.tensor.matmul(
                        acc[:, c, :],
                        lhsT=E[g][h][:, jt, it * 128:(it + 1) * 128],
                        rhs=Vt[jt][:, h, :],
                        start=(jt == 0),
                        stop=(jt == JT - 1),
                    )
            rec = rec_pool.tile([128, 2], F32, tag="rec", name="rec")
            nc.vector.reciprocal(rec, acc[:, :, DH])
            ona = onat_pool.tile([128, 2, DH], BF16, tag="onat", name="ona")
            for c in range(2):
                nc.vector.tensor_scalar_mul(
                    ona[:, c, :], acc[:, c, 0:DH], rec[:, c:c + 1]
                )
            onat.setdefault(g, {})[it] = ona

        def o_tr(g, it, act_copy=False):
            # transpose [128 i, 128 hd] -> OT[g] columns for this i-tile
            tp = o_ps.tile([128, 128], BF16, tag="ops", name="tp")
            nc.tensor.transpose(
                tp, onat[g][it].rearrange("p a b -> p (a b)"), ident
            )
            if act_copy:
                # ACT is idle in the tail; Copy stays in the loaded table set
                nc.scalar.activation(
                    OT[g][:, it * 128:(it + 1) * 128], tp,
                    mybir.ActivationFunctionType.Copy,
                )
            else:
                nc.vector.tensor_copy(OT[g][:, it * 128:(it + 1) * 128], tp)
            if it == TT - 1:
                del onat[g]
                del E[g]

        pj_partial = [
            pjp_pool.tile([128, C], BF16, tag="pjpart", name=f"pjp_{i}")
            for i in range(TT)
        ]

        def pj1(tt):
            # head pairs 0-4: overlaps the ACT-bound tail of attention pair 5
            for oc in range(2):
                acc = qkv_ps.tile([128, 384], F32, tag="qkvps", name="pj_acc")
                for g in range(5):
                    nc.tensor.matmul(
                        acc,
                        lhsT=OT[g][:, tt * 128:(tt + 1) * 128],
                        rhs=wp_all[:, g, oc * 384:(oc + 1) * 384],
                        start=(g == 0),
                        stop=(g == 4),
                    )
                nc.vector.tensor_add(
                    pj_partial[tt][:, oc * 384:(oc + 1) * 384],
                    acc,
                    bias_bc[:, oc * 384:(oc + 1) * 384],
                )

        def pj2(tt):
            # head pair 5 only - the thinnest possible serial tail
            osb = out_pool.tile([128, C], F32, tag="outsb", name="osb")
            for oc in range(2):
                acc = qkv_ps.tile([128, 384], F32, tag="qkvps", name="pj_acc")
                nc.tensor.matmul(
                    acc,
                    lhsT=OT[5][:, tt * 128:(tt + 1) * 128],
                    rhs=wp_all[:, 5, oc * 384:(oc + 1) * 384],
                    start=True,
                    stop=True,
                )
                nc.vector.tensor_add(
                    osb[:, oc * 384:(oc + 1) * 384],
                    acc,
                    pj_partial[tt][:, oc * 384:(oc + 1) * 384],
                )
                deng = (nc.scalar, nc.gpsimd, nc.sync)[(2 * tt + oc) % 3]
                deng.dma_start(
                    out=out_d[tt * 128:(tt + 1) * 128, oc * 384:(oc + 1) * 384],
                    in_=osb[:, oc * 384:(oc + 1) * 384],
                )

        # ---------------- emission schedule ----------------
        # Steady-state slot p (pairs 1..4): S fills of pair p (ACT-paced via
        # the 2 sps bufs), O ic1 of pair p-1, QKV chains of pair p+1, O ic0
        # of pair p, transposes of pair p-1 - all woven at unit granularity.
        def s_units(g, ic):
            return [(s_unit, g, ic, u, h) for u in range(4) for h in (2 * g, 2 * g + 1)]

        def weave(a, b):
            # interleave pacer list a with filler list b, spreading b evenly
            # (fractional accumulator) so no burst of fillers stalls on PSUM
            out = []
            ib = 0
            acc_f = 0.0
            step = len(b) / max(1, len(a))
            for ia, item in enumerate(a):
                out.append(item)
                acc_f += step
                while ib < round(acc_f):
                    out.append(b[ib]); ib += 1
            out.extend(b[ib:])
            return out

        for _ in range(repeat):
            sched = []
            # startup: pair-0 weights and x in parallel across the DMA queues
            sched += [(load_x, 0)]
            sched += [(load_w, 0)]
            sched += [(load_x, rt) for rt in range(1, CT)]
            sched += [(load_w, 6), (load_w, 12), (load_w, 13), (load_w, 14)]
            sched += [(load_bias,)]
            sched += [(qk_chain, 0, w) for w in range(4)]
            # pair-0 S fills woven with v chains (exp(0) streams on ACT)
            sched += weave(
                s_units(0, 0) + s_units(0, 1),
                [(v_chain, 0, tt) for tt in range(TT)]
                + [(load_w, 1), (load_w, 7)]
                + [(qk_chain, 1, w) for w in range(4)]
                + [(o_acc, 0, i) for i in range(4)]
                + [(o_tr, 0, i) for i in range(4)],
            )
            # slots for pairs 1..4
            for p in range(1, 5):
                filler = [(o_acc, p - 1, 4 + i) for i in range(4)]
                filler += [(o_tr, p - 1, 4 + i) for i in range(4)]
                if p == 2:
                    filler += [(load_w, 15), (load_w, 16), (load_w, 17)]
                filler += [(load_w, p + 1), (load_w, 7 + p)]
                filler += [(qk_chain, p + 1, w) for w in range(4)]
                if p == 4:
                    filler += [(load_wp, rt) for rt in range(CT)]
                if p == 3:
                    filler += [(v_chain, 1, tt) for tt in range(TT)]
                filler += [(o_acc, p, i) for i in range(4)]
                filler += [(o_tr, p, i) for i in range(4)]
                sched += weave(s_units(p, 0) + s_units(p, 1), filler)
            # slot for pair 5: S fills + O(4) + proj pass 1
            filler = [(o_acc, 4, 4 + i) for i in range(4)]
            filler += [(o_tr, 4, 4 + i) for i in range(4)]
            filler += [(pj1, tt) for tt in range(TT)]
            filler += [(o_acc, 5, i) for i in range(4)]
            filler += [(o_tr, 5, i) for i in range(4)]
            sched += weave(s_units(5, 0) + s_units(5, 1), filler)
            # tail: weave O ic1 of pair 5 between the ready ic0 transposes and
            # proj-pass-2 tiles so every engine always has a ready item
            for it in range(TT):
                if it >= 4:
                    sched += [(o_tr, 5, it, True)]
                sched += [(pj2, it)]
                if it < 4:
                    sched += [(o_acc, 5, 4 + it)]

            for item in sched:
                item[0](*item[1:])


_NC_CACHE = None


def _get_nc():
    global _NC_CACHE
    if _NC_CACHE is None:
        _NC_CACHE = _build_nc()
    return _NC_CACHE


def _prep_in_maps(x, qkv_w, proj_w, proj_b):
    """Host-side shard + pre-transpose + bf16 cast. Returns per-core in_maps."""
    import ml_dtypes

    BF = ml_dtypes.bfloat16
    x = np.ascontiguousarray(np.asarray(x, dtype=np.float32))
    qkv_w = np.asarray(qkv_w, dtype=np.float32)
    proj_w = np.asarray(proj_w, dtype=np.float32)
    proj_b = np.ascontiguousarray(np.asarray(proj_b, dtype=np.float32))

    xf = x.reshape(B, N, C)
    xt = np.ascontiguousarray(xf.transpose(0, 2, 1).astype(BF))          # [B, C, N]
    qkv_wt = np.ascontiguousarray(
        qkv_w.reshape(WT, 128, CT, 128).transpose(0, 3, 2, 1).astype(BF)
    ).reshape(WT, 128, C)
    proj_wt = np.ascontiguousarray(
        proj_w.reshape(CT, 128, CT, 128).transpose(0, 3, 2, 1).astype(BF)
    ).reshape(CT, 128, C)
    return [
        {"xT": xt[i], "qkv_wt": qkv_wt, "proj_wt": proj_wt, "proj_b": proj_b}
        for i in range(NCORES)
    ]


def kernel(x, qkv_w, proj_w, proj_b, _trace=False):
    from concourse.bass_utils import run_bass_kernel_spmd

    in_maps = _prep_in_maps(x, qkv_w, proj_w, proj_b)
    nc = _get_nc()
    res = run_bass_kernel_spmd(nc, in_maps, core_ids=list(range(NCORES)), trace=_trace)
    out = np.stack([r["out"] for r in res.results], axis=0).reshape(B, 32, 32, C)
    if _trace:
        kernel._last_results = res
    return out
